# revision 8
# baseline (speedup 1.0000x reference)
"""Grouped Conv2d (512 groups, 2->2 ch/group, 3x3 VALID) on 8 trn2 NeuronCores.

Strategy:
  - Shard the 512 groups across 8 cores: 64 groups = 128 channels per core,
    which exactly fills the 128 SBUF partitions. Fully independent (no
    collectives); batch stays whole on every core.
  - On-device compute: for each 3x3 tap (kh,kw) build a 128x128
    block-diagonal weight matrix (64 blocks of 2x2) host-side; the grouped
    conv then becomes 9 accumulating PE matmuls per output tile:
        psum[oc, i, j] += W_tap[ic, oc]^T . x[ic, i+kh, j+kw]
    run in float32r (full-rate fp32 on the PE for moving dim >= 256).
  - Output rows are produced in 6 chunks of 9 rows (9*54 = 486 <= 512 fp32
    = one PSUM bank), evicted PSUM->SBUF on alternating scalar/vector
    engines, and DMA'd back per batch.
"""

import sys

import numpy as np

for _p in ("/opt/trn_rl_repo",):
    if _p not in sys.path:
        sys.path.insert(0, _p)

import concourse.bacc as bacc
import concourse.bass as bass
import concourse.tile as tile
from concourse import mybir
from concourse.bass_utils import run_bass_kernel_spmd

N_CORES = 8
B, C, H, W = 16, 1024, 56, 56
KH = KW = 3
HO, WO = H - KH + 1, W - KW + 1  # 54, 54
CPC = C // N_CORES  # 128 channels (64 groups) per core
ROWS_PER_CHUNK = 9  # 9*54 = 486 fp32 <= 512 (one PSUM bank)
N_CHUNKS = HO // ROWS_PER_CHUNK  # 6

_NC_CACHE = None


def _build_program():
    nc = bacc.Bacc(
        "TRN2", target_bir_lowering=False, debug=False, num_devices=N_CORES
    )
    f32 = mybir.dt.float32
    f32r = mybir.dt.float32r

    x_d = nc.declare_dram_parameter("x", [B, CPC, H, W], f32, isOutput=False)
    wm_d = nc.declare_dram_parameter(
        "wm", [CPC, KH * KW, CPC], f32, isOutput=False
    )
    y_d = nc.declare_dram_parameter("y", [B, CPC, HO, WO], f32, isOutput=True)

    with tile.TileContext(nc) as tc:
        with (
            tc.tile_pool(name="wpool", bufs=1) as wpool,
            tc.tile_pool(name="xpool", bufs=3) as xpool,
            tc.tile_pool(name="opool", bufs=3) as opool,
            tc.tile_pool(name="psum", bufs=6, space="PSUM") as ppool,
            tc.tile_pool(name="scratch", bufs=1, space="PSUM") as spool,
        ):
            wt = wpool.tile([CPC, KH * KW, CPC], f32r)
            nc.sync.dma_start(out=wt[:], in_=wm_d[:].bitcast(f32r))

            # The fused f32r matmul (LDW+MM) supports only ONE semaphore
            # wait; Tile would otherwise put {wt-DMA, xt-DMA} (2 waits) on
            # the first matmul of each batch. These 1-column "sync" matmuls
            # absorb the DMA waits; PE program order covers the rest.
            scr = spool.tile([CPC, 2], f32)
            nc.tensor.matmul(
                scr[:], lhsT=wt[:, 0, :], rhs=wt[:, 0, :2],
                start=True, stop=True,
            )

            for n in range(B):
                xt = xpool.tile([CPC, H, W], f32r)
                nc.sync.dma_start(out=xt[:], in_=x_d[n].bitcast(f32r))
                nc.tensor.matmul(
                    scr[:], lhsT=wt[:, 0, :], rhs=xt[:, 0, :2],
                    start=True, stop=True,
                )
                ot = opool.tile([CPC, HO, WO], f32)
                for c in range(N_CHUNKS):
                    r0 = c * ROWS_PER_CHUNK
                    pt = ppool.tile([CPC, ROWS_PER_CHUNK, WO], f32)
                    t = 0
                    for kh in range(KH):
                        for kw in range(KW):
                            nc.tensor.matmul(
                                pt[:],
                                lhsT=wt[:, t, :],
                                rhs=xt[
                                    :,
                                    r0 + kh : r0 + kh + ROWS_PER_CHUNK,
                                    kw : kw + WO,
                                ],
                                start=(t == 0),
                                stop=(t == KH * KW - 1),
                            )
                            t += 1
                    dst = ot[:, r0 : r0 + ROWS_PER_CHUNK, :]
                    if c % 2 == 0:
                        nc.scalar.activation(
                            dst, pt[:], mybir.ActivationFunctionType.Copy
                        )
                    else:
                        nc.vector.tensor_copy(dst, pt[:])
                nc.sync.dma_start(out=y_d[n], in_=ot[:])
    nc.compile()
    return nc


def _get_nc():
    global _NC_CACHE
    if _NC_CACHE is None:
        _NC_CACHE = _build_program()
    return _NC_CACHE


def _make_wmats(w):
    """Per-core lhsT weight mats, shape (128, 9, 128): wm[ic, t, oc]."""
    oc = np.arange(CPC)
    mats = []
    for cid in range(N_CORES):
        ws = np.asarray(w[cid * CPC : (cid + 1) * CPC], dtype=np.float32)
        wm = np.zeros((CPC, KH * KW, CPC), dtype=np.float32)
        for icg in range(2):
            ic = (oc // 2) * 2 + icg
            # advanced indexing on dims 0 and 2 -> result dims (pair, tap)
            wm[ic, :, oc] = ws[oc, icg].reshape(CPC, KH * KW)
        mats.append(wm)
    return mats


def _run(x, w, trace=False, **kwargs):
    nc = _get_nc()
    x = np.asarray(x, dtype=np.float32)
    wmats = _make_wmats(w)
    in_maps = [
        {
            "x": np.ascontiguousarray(x[:, cid * CPC : (cid + 1) * CPC]),
            "wm": wmats[cid],
        }
        for cid in range(N_CORES)
    ]
    res = run_bass_kernel_spmd(
        nc, in_maps, list(range(N_CORES)), trace=trace, **kwargs
    )
    y = np.concatenate(
        [res.results[i]["y"] for i in range(N_CORES)], axis=1
    )
    return y, res


def kernel(x, w):
    y, _ = _run(x, w, trace=False)
    return y
